# revision 1
# baseline (speedup 1.0000x reference)
"""Trainium2 Bass kernel for nn_CausalSelfAttention (modded-nanogpt quantized attention).

Sharding: 8 cores = 2 batches x 4 head-groups (2 heads each).
Each core computes QKV for its 2 heads from the full x[b], runs causal
attention + gating for those heads, and produces a partial output
projection (its 256 feature columns of w_o); the host sums the 4 partials
per batch. Weight ternary-quantization scales (4 global scalars) are
computed host-side; everything else runs on device.

Key device-side structure per core:
 - x[b] int8-fake-quantized per-token in natural [t,d] tiles (per-partition
   scales, magic-constant RNE round), then PE-transposed into xqT [d,t]
   blocks for the QKV matmul (ternary weights, global scale folded out).
 - q,k: rmsnorm folded into the quant scales (alpha fold), rotary, int8
   fake-quant in natural [t, head*hd] tiles; PE-transpose to [hd, t].
 - scores computed transposed S_T[tk, tq] = kk_T.T @ qq_T so softmax needs
   no transposes: exp on ACT (scale=0.12 fused), denominator via a ones
   column appended to v, y = E.T @ [v|1] accumulated in PSUM over tk.
 - softmax without max-subtraction (|scores| <= 0.12*128 => exp safe).
 - gate = sigmoid(xq[:, :12] @ gw.T) * s_o / den folded into one per-token
   scalar applied to y; output projection from PE-transposed y.
"""

import numpy as np

B, T, DIM, H, HD = 2, 2048, 1024, 8, 128
ATTN_SCALE = 0.12
F32_EPS = float(np.finfo(np.float32).eps)
MAGIC = float(np.float32(1.5 * 2 ** 23))  # RNE round for |x| < 2^22
NT = T // 128          # 16 t-tiles
ND = DIM // 128        # 8 d-tiles
HLOC = 2               # heads per core
ELOC = HLOC * HD       # 256 local features
NSTRIP = T // 512      # 4 tq strips per head

_CACHE = {}
DEBUG = False
PHASE = 4  # 1=xq 2=+qkv/chains 3=+attention 4=full
SPLIT_DLOOP = False   # sequential qk/v accumulation loops instead of interleaved
NO_TTR = True         # custom tensor_tensor_reduce DVE op fails on this runtime
NO_GATE_MM = False    # skip k=12 gate matmul


def _build():
    import concourse.bass as bass
    import concourse.mybir as mybir
    import concourse.tile as tile
    from concourse import bacc
    from concourse.masks import make_identity, make_upper_triangular
    from contextlib import ExitStack

    f32 = mybir.dt.float32
    A = mybir.AluOpType
    AF = mybir.ActivationFunctionType
    X = mybir.AxisListType.X

    nc = bacc.Bacc(trn_type="TRN2")

    # extra activation-bias constants (Bass pre-registers only 0.0/1.0)
    for _v in (MAGIC, -(MAGIC - 1.0), 2.0):
        _t = nc.alloc_sbuf_tensor(f"const-float32-{_v}", [128, 1], f32)
        nc.gpsimd.memset(_t.ap(), _v)
        nc.const_aps.aps[(f32, _v)] = _t.ap()
    nc.all_engine_barrier()

    xb = nc.dram_tensor("xb", [T, DIM], f32, kind="ExternalInput")
    veb = nc.dram_tensor("veb", [T, ELOC], f32, kind="ExternalInput")
    cos2 = nc.dram_tensor("cos2", [T, ELOC], f32, kind="ExternalInput")
    sin2 = nc.dram_tensor("sin2", [T, ELOC], f32, kind="ExternalInput")
    wqkvT = nc.dram_tensor("wqkvT", [DIM, 3 * ELOC], f32, kind="ExternalInput")
    woTq = nc.dram_tensor("woTq", [ELOC, DIM], f32, kind="ExternalInput")
    gwT = nc.dram_tensor("gwT", [12, HLOC], f32, kind="ExternalInput")
    # scal cols: s_q, s_k, s_v, s_o, inv_s_q, inv_s_k, inv_s_v, inv_s_o
    scal = nc.dram_tensor("scal", [128, 8], f32, kind="ExternalInput")
    lam = nc.dram_tensor("lam", [128, 2], f32, kind="ExternalInput")
    outp = nc.dram_tensor("outp", [T, DIM], f32, kind="ExternalOutput")
    if DEBUG:
        dbg_xq = nc.dram_tensor("dbg_xq", [T, DIM], f32, kind="ExternalOutput")
        dbg_q = nc.dram_tensor("dbg_q", [T, ELOC], f32, kind="ExternalOutput")
        dbg_k = nc.dram_tensor("dbg_k", [T, ELOC], f32, kind="ExternalOutput")
        dbg_al = nc.dram_tensor("dbg_al", [T, 2 * HLOC], f32, kind="ExternalOutput")
        dbg_g = nc.dram_tensor("dbg_g", [T, HLOC], f32, kind="ExternalOutput")

    with tile.TileContext(nc) as tc, ExitStack() as ctx:
        singles = ctx.enter_context(tc.tile_pool(name="singles", bufs=1))
        xpool = ctx.enter_context(tc.tile_pool(name="xpool", bufs=2))
        qkpool = ctx.enter_context(tc.tile_pool(name="qkpool", bufs=2))
        scl = ctx.enter_context(tc.tile_pool(name="scl", bufs=4))
        epool = ctx.enter_context(tc.tile_pool(name="epool", bufs=4))
        ypool = ctx.enter_context(tc.tile_pool(name="ypool", bufs=4))
        opool = ctx.enter_context(tc.tile_pool(name="opool", bufs=2))
        psA = ctx.enter_context(tc.tile_pool(name="psA", bufs=2, space="PSUM"))
        psB = ctx.enter_context(tc.tile_pool(name="psB", bufs=2, space="PSUM"))
        psC = ctx.enter_context(tc.tile_pool(name="psC", bufs=4, space="PSUM"))

        def ts(out, in0, s1, s2=None, op0=A.mult, op1=None, eng=None):
            e = eng if eng is not None else nc.any
            kw = {}
            if op1 is not None:
                kw["op1"] = op1
            e.tensor_scalar(out=out, in0=in0, scalar1=s1, scalar2=s2, op0=op0, **kw)

        # ---------------- constants / small inputs ----------------
        ident = singles.tile([128, 128], f32)
        make_identity(nc, ident)
        trilE = singles.tile([128, 128], f32)  # E.T diag mask: keep tk<=tq
        make_upper_triangular(nc, trilE, val=1.0, diag=True)

        scal_sb = singles.tile([128, 8], f32)
        nc.sync.dma_start(out=scal_sb, in_=scal[:, :])
        lam_sb = singles.tile([128, 2], f32)
        nc.sync.dma_start(out=lam_sb, in_=lam[:, :])
        gw_sb = singles.tile([12, HLOC], f32)
        nc.sync.dma_start(out=gw_sb, in_=gwT[:, :])

        lam0sv = singles.tile([128, 1], f32)
        ts(lam0sv, lam_sb[:, 0:1], scal_sb[:, 2:3], eng=nc.vector)
        sq2 = singles.tile([128, 2], f32)  # s_q^2, s_k^2
        for j in range(2):
            ts(sq2[:, j:j + 1], scal_sb[:, j:j + 1], scal_sb[:, j:j + 1], eng=nc.vector)

        # -------- weights: load + ternary quantize (global scale folded out) ----
        tau = singles.tile([128, ND, 3 * ELOC], f32)
        nc.sync.dma_start(out=tau, in_=wqkvT.rearrange("(n p) e -> p n e", p=128))
        for s, weng in ((0, nc.vector), (1, None), (2, nc.gpsimd)):
            w = tau[:, :, s * ELOC:(s + 1) * ELOC]
            if weng is None:  # ACT chain (exact: same two roundings + int clip)
                nc.scalar.activation(w, w, AF.Identity, bias=MAGIC,
                                     scale=scal_sb[:, 4 + s:5 + s])
                nc.scalar.activation(w, w, AF.Relu, bias=-(MAGIC - 1.0))
                nc.scalar.activation(w, w, AF.Relu, scale=-1.0, bias=2.0)
                nc.scalar.activation(w, w, AF.Identity, scale=-1.0, bias=1.0)
            else:
                ts(w, w, scal_sb[:, 4 + s:5 + s], MAGIC, A.mult, A.add, eng=weng)
                ts(w, w, MAGIC, -1.0, A.subtract, A.max, eng=weng)
                ts(w, w, 1.0, None, A.min, eng=weng)
        tau_o = singles.tile([128, HLOC, DIM], f32)
        nc.sync.dma_start(out=tau_o, in_=woTq.rearrange("(n p) e -> p n e", p=128))
        ts(tau_o, tau_o, scal_sb[:, 7:8], MAGIC, A.mult, A.add, eng=nc.vector)
        ts(tau_o, tau_o, MAGIC, -1.0, A.subtract, A.max, eng=nc.vector)
        ts(tau_o, tau_o, 1.0, None, A.min, eng=nc.vector)

        # ---------------- persistent activations ----------------
        qT = singles.tile([128, HLOC, T], f32)   # [hd, h, t] quantized q
        kT = singles.tile([128, HLOC, T], f32)
        vaug = singles.tile([128, HLOC, NT, HD + 1], f32)  # [tk, h, tile, hd|1]
        nc.gpsimd.memset(vaug[:, :, :, HD:HD + 1], 1.0)
        gate_so = singles.tile([128, NT, HLOC], f32)       # sigmoid(gate)*s_o
        yT = singles.tile([128, HLOC, T], f32)             # [hd, h, t] gated y

        # ======== per t-tile: x quant -> xqT -> QKV -> q/k chain -> v ========
        for i in range(NT):
            xt = xpool.tile([128, DIM], f32, tag="xt")
            nc.sync.dma_start(out=xt, in_=xb[i * 128:(i + 1) * 128, :])

            # per-token scales
            mx = scl.tile([128, 1], f32, tag="mx")
            mn = scl.tile([128, 1], f32, tag="mn")
            nc.vector.tensor_reduce(out=mx, in_=xt, axis=X, op=A.max)
            nc.vector.tensor_reduce(out=mn, in_=xt, axis=X, op=A.min)
            ts(mx, mx, 1e-5, None, A.max, eng=nc.vector)
            ts(mn, mn, -1e-5, None, A.min, eng=nc.vector)
            mp = scl.tile([128, 1], f32, tag="mp")
            mnn = scl.tile([128, 1], f32, tag="mnn")
            nc.vector.reciprocal(out=mp, in_=mx)
            nc.vector.reciprocal(out=mnn, in_=mn)
            ts(mp, mp, 127.0, eng=nc.vector)
            ts(mnn, mnn, 127.0, eng=nc.vector)
            sp = scl.tile([128, 1], f32, tag="sp")
            sn = scl.tile([128, 1], f32, tag="sn")
            ts(sp, mx, 1.0 / 127.0, eng=nc.vector)
            ts(sn, mn, 1.0 / 127.0, eng=nc.vector)

            # two-branch quant: xq = (rp-M)*sp + (rn-M)*sn
            zp = xpool.tile([128, DIM], f32, tag="zp")
            zn = xpool.tile([128, DIM], f32, tag="zn")
            nc.scalar.activation(zp, xt, AF.Relu, scale=mp)       # max(x,0)*mp
            nc.scalar.activation(zp, zp, AF.Identity, bias=MAGIC)  # + M (RNE round)
            ts(zn, xt, 0.0, mnn, A.min, A.mult, eng=nc.gpsimd)
            ts(zn, zn, MAGIC, None, A.add, eng=nc.gpsimd)
            xq = xpool.tile([128, DIM], f32, tag="xq")
            ts(xq, zp, MAGIC, sp, A.subtract, A.mult, eng=nc.vector)
            ts(zn, zn, MAGIC, sn, A.subtract, A.mult, eng=nc.gpsimd)
            nc.vector.tensor_tensor(out=xq, in0=xq, in1=zn, op=A.add)

            if DEBUG:
                nc.sync.dma_start(out=dbg_xq[i * 128:(i + 1) * 128, :], in_=xq)
            if PHASE == 1:
                nc.sync.dma_start(out=outp[i * 128:(i + 1) * 128, :], in_=xq)
                continue
            # transpose xq -> xqT block [128(d), ND, 128(t)]
            xqT = xpool.tile([128, ND, 128], f32, tag="xqT")
            for g in range(2):
                ps = psA.tile([128, 4, 128], f32, tag="a")
                for d4 in range(4):
                    d = 4 * g + d4
                    nc.tensor.transpose(ps[:, d4, :], xq[:, d * 128:(d + 1) * 128], ident)
                nc.any.tensor_copy(out=xqT[:, 4 * g:4 * g + 4, :], in_=ps)

            # gate logits: lhsT = xqT[0:12, 0, :] (quantized x.T rows 0..11)
            if NO_GATE_MM:
                nc.vector.memset(gate_so[:, i, :], 0.5)
            else:
                gps = psC.tile([128, HLOC], f32, tag="c")
                nc.tensor.matmul(gps[0:128, :], xqT[0:12, 0, :], gw_sb, start=True, stop=True)
                nc.scalar.activation(gate_so[:, i, :], gps, AF.Sigmoid)
                ts(gate_so[:, i, :], gate_so[:, i, :], scal_sb[:, 3:4], eng=nc.vector)

            if DEBUG:
                nc.sync.dma_start(out=dbg_g[i * 128:(i + 1) * 128, :], in_=gate_so[:, i, :])
            # QKV matmuls for this t-tile
            qk_ps = psB.tile([128, 2 * ELOC], f32, tag="b")
            v_ps = psC.tile([128, ELOC], f32, tag="c")
            if SPLIT_DLOOP:
                for d in range(ND):
                    nc.tensor.matmul(qk_ps, xqT[:, d, :], tau[:, d, 0:2 * ELOC],
                                     start=(d == 0), stop=(d == ND - 1))
                for d in range(ND):
                    nc.tensor.matmul(v_ps, xqT[:, d, :], tau[:, d, 2 * ELOC:3 * ELOC],
                                     start=(d == 0), stop=(d == ND - 1))
            else:
                for d in range(ND):
                    nc.tensor.matmul(qk_ps, xqT[:, d, :], tau[:, d, 0:2 * ELOC],
                                     start=(d == 0), stop=(d == ND - 1))
                    nc.tensor.matmul(v_ps, xqT[:, d, :], tau[:, d, 2 * ELOC:3 * ELOC],
                                     start=(d == 0), stop=(d == ND - 1))

            # ---- v mix into vaug ----
            vet = xpool.tile([128, ELOC], f32, tag="vet")
            nc.sync.dma_start(out=vet, in_=veb[i * 128:(i + 1) * 128, :])
            ts(vet, vet, lam_sb[:, 1:2])
            for h in range(HLOC):
                nc.vector.scalar_tensor_tensor(
                    out=vaug[:, h, i, 0:HD], in0=v_ps[:, h * HD:(h + 1) * HD],
                    scalar=lam0sv, in1=vet[:, h * HD:(h + 1) * HD],
                    op0=A.mult, op1=A.add)

            # rotary inputs for this tile
            cost = xpool.tile([128, ELOC], f32, tag="cost")
            sint = xpool.tile([128, ELOC], f32, tag="sint")
            nc.sync.dma_start(out=cost, in_=cos2[i * 128:(i + 1) * 128, :])
            nc.sync.dma_start(out=sint, in_=sin2[i * 128:(i + 1) * 128, :])

            # ---- q/k chains ----
            for scol, dstT in ((0, qT), (1, kT)):
                off = scol * ELOC
                nat = qkpool.tile([128, ELOC], f32, tag="nat")
                nc.any.tensor_copy(out=nat, in_=qk_ps[:, off:off + ELOC])

                # alpha per head (rms fold, exact eps handling)
                al = scl.tile([128, HLOC], f32, tag="al")
                for h in range(HLOC):
                    junk = qkpool.tile([128, HD], f32, tag="junk")
                    ssq = scl.tile([128, 1], f32, tag="ssq")
                    if NO_TTR:
                        sqeng = nc.vector if scol == 0 else nc.gpsimd
                        sqeng.tensor_tensor(out=junk, in0=nat[:, h * HD:(h + 1) * HD],
                                            in1=nat[:, h * HD:(h + 1) * HD], op=A.mult)
                        nc.vector.tensor_reduce(out=ssq, in_=junk, axis=X, op=A.add)
                    else:
                        nc.vector.tensor_tensor_reduce(
                            out=junk, in0=nat[:, h * HD:(h + 1) * HD],
                            in1=nat[:, h * HD:(h + 1) * HD], scale=1.0,
                            scalar=0.0, op0=A.mult, op1=A.add, accum_out=ssq)
                    nc.vector.scalar_tensor_tensor(out=ssq, in0=ssq, scalar=1.0 / HD,
                                                   in1=sq2[:, scol:scol + 1],
                                                   op0=A.mult, op1=A.mult)
                    ts(ssq, ssq, F32_EPS, None, A.add, eng=nc.vector)
                    nc.scalar.activation(ssq, ssq, AF.Sqrt)
                    nc.vector.reciprocal(out=al[:, h:h + 1], in_=ssq)
                    ts(al[:, h:h + 1], al[:, h:h + 1], scal_sb[:, scol:scol + 1],
                       eng=nc.vector)

                # rotary (on unnormalized values; alpha folded into quant scales)
                reng = nc.vector if scol == 0 else nc.gpsimd
                n3 = nat.rearrange("p (h d) -> p h d", h=HLOC)
                rot = qkpool.tile([128, ELOC], f32, tag="rot")
                r3 = rot.rearrange("p (h d) -> p h d", h=HLOC)
                t2 = qkpool.tile([128, ELOC], f32, tag="t2")
                t3 = t2.rearrange("p (h d) -> p h d", h=HLOC)
                s3 = sint.rearrange("p (h d) -> p h d", h=HLOC)
                reng.tensor_tensor(out=rot, in0=nat, in1=cost, op=A.mult)
                reng.tensor_tensor(out=t3[:, :, 0:64], in0=n3[:, :, 64:128],
                                   in1=s3[:, :, 0:64], op=A.mult)
                reng.tensor_tensor(out=t3[:, :, 64:128], in0=n3[:, :, 0:64],
                                   in1=s3[:, :, 64:128], op=A.mult)
                reng.tensor_tensor(out=rot, in0=rot, in1=t2, op=A.add)

                # min/max per head
                mx2 = scl.tile([128, HLOC], f32, tag="mx2")
                mn2 = scl.tile([128, HLOC], f32, tag="mn2")
                nc.vector.tensor_reduce(out=mx2, in_=r3, axis=X, op=A.max)
                nc.vector.tensor_reduce(out=mn2, in_=r3, axis=X, op=A.min)

                qq = qkpool.tile([128, ELOC], f32, tag="qq")
                for h in range(HLOC):
                    hs = slice(h * HD, (h + 1) * HD)
                    ceng = nc.gpsimd if (scol == 1 and h == 1) else nc.vector
                    xpm = scl.tile([128, 1], f32, tag="xpm")
                    xnm = scl.tile([128, 1], f32, tag="xnm")
                    ts(xpm, mx2[:, h:h + 1], 1e-5, None, A.max, eng=nc.vector)
                    ts(xnm, mn2[:, h:h + 1], -1e-5, None, A.min, eng=nc.vector)
                    mp2 = scl.tile([128, 1], f32, tag="mp2")
                    mn2_ = scl.tile([128, 1], f32, tag="mn2_")
                    nc.vector.reciprocal(out=mp2, in_=xpm)
                    nc.vector.reciprocal(out=mn2_, in_=xnm)
                    ts(mp2, mp2, 127.0, eng=nc.vector)
                    ts(mn2_, mn2_, 127.0, eng=nc.vector)
                    sp2 = scl.tile([128, 1], f32, tag="sp2")
                    sn2 = scl.tile([128, 1], f32, tag="sn2")
                    nc.vector.scalar_tensor_tensor(out=sp2, in0=xpm, scalar=1.0 / 127.0,
                                                   in1=al[:, h:h + 1], op0=A.mult, op1=A.mult)
                    nc.vector.scalar_tensor_tensor(out=sn2, in0=xnm, scalar=1.0 / 127.0,
                                                   in1=al[:, h:h + 1], op0=A.mult, op1=A.mult)
                    zp2 = qkpool.tile([128, HD], f32, tag="zp2")
                    zn2 = qkpool.tile([128, HD], f32, tag="zn2")
                    ts(zp2, r3[:, h, :], 0.0, mp2, A.max, A.mult, eng=ceng)
                    ts(zn2, r3[:, h, :], 0.0, mn2_, A.min, A.mult, eng=ceng)
                    ts(zp2, zp2, MAGIC, None, A.add, eng=ceng)
                    ts(zn2, zn2, MAGIC, None, A.add, eng=ceng)
                    ts(qq[:, hs], zp2, MAGIC, sp2, A.subtract, A.mult, eng=ceng)
                    ts(zn2, zn2, MAGIC, sn2, A.subtract, A.mult, eng=ceng)
                    ceng.tensor_tensor(out=qq[:, hs], in0=qq[:, hs], in1=zn2, op=A.add)

                if DEBUG:
                    dbg_t = dbg_q if scol == 0 else dbg_k
                    nc.sync.dma_start(out=dbg_t[i * 128:(i + 1) * 128, :], in_=qq)
                    nc.sync.dma_start(
                        out=dbg_al[i * 128:(i + 1) * 128, scol * HLOC:(scol + 1) * HLOC],
                        in_=al)
                if PHASE == 2:
                    nc.sync.dma_start(
                        out=outp[i * 128:(i + 1) * 128, scol * ELOC:(scol + 1) * ELOC],
                        in_=qq)
                    continue
                # transpose qq -> dstT[:, h, i*128:(i+1)*128]
                for h in range(HLOC):
                    psq = psA.tile([128, 4, 128], f32, tag="a")
                    nc.tensor.transpose(psq[:, 0, :], qq[:, h * HD:(h + 1) * HD], ident)
                    nc.any.tensor_copy(out=dstT[:, h, i * 128:(i + 1) * 128],
                                       in_=psq[:, 0, :])

        # ======== attention per head, per tq strip ========
        for h in (range(HLOC) if PHASE >= 3 else []):
            for J in range(NSTRIP):
                yu0 = psC.tile([128, HD + 1], f32, tag="c")
                yu1 = psC.tile([128, HD + 1], f32, tag="c")
                yu2 = psC.tile([128, HD + 1], f32, tag="c")
                yu3 = psC.tile([128, HD + 1], f32, tag="c")
                yu = [yu0, yu1, yu2, yu3]
                for i in range(4 * J + 4):
                    st = psA.tile([128, 4, 128], f32, tag="a")
                    stf = st.rearrange("p a b -> p (a b)")
                    nc.tensor.matmul(stf, kT[:, h, i * 128:(i + 1) * 128],
                                     qT[:, h, J * 512:(J + 1) * 512],
                                     start=True, stop=True)
                    lo = max(0, 128 * (i - 4 * J))
                    E = epool.tile([128, 512], f32, tag="E")
                    nc.scalar.activation(E[:, lo:512], stf[:, lo:512], AF.Exp,
                                         scale=ATTN_SCALE)
                    if i >= 4 * J:
                        dl = 128 * (i - 4 * J)
                        # keep tk<=tq: out[x,y] = (y - x) >= 0 ? E : 0
                        nc.gpsimd.affine_select(
                            out=E[:, dl:dl + 128], in_=E[:, dl:dl + 128],
                            compare_op=A.is_ge, fill=0.0, base=0,
                            pattern=[[1, 128]], channel_multiplier=-1)
                    for j in range(max(4 * J, i), 4 * J + 4):
                        jj = j - 4 * J
                        nc.tensor.matmul(yu[jj][:, :],
                                         E[:, jj * 128:(jj + 1) * 128],
                                         vaug[:, h, i, :],
                                         start=(i == 0), stop=(i == j))
                # normalize + gate -> y natural, transpose into yT
                for jj in range(4):
                    j = 4 * J + jj
                    den = scl.tile([128, 1], f32, tag="den")
                    nc.vector.reciprocal(out=den, in_=yu[jj][:, HD:HD + 1])
                    gam = scl.tile([128, 1], f32, tag="gam")
                    nc.vector.tensor_tensor(out=gam, in0=den,
                                            in1=gate_so[:, j, h:h + 1], op=A.mult)
                    ynat = ypool.tile([128, HD], f32, tag="ynat")
                    ts(ynat, yu[jj][:, 0:HD], gam)
                    psy = psA.tile([128, 4, 128], f32, tag="a")
                    nc.tensor.transpose(psy[:, 0, :], ynat, ident)
                    nc.any.tensor_copy(out=yT[:, h, j * 128:(j + 1) * 128],
                                       in_=psy[:, 0, :])

        # ======== output projection (partial: this core's 256 features) ========
        for i in (range(NT) if PHASE >= 4 else []):
            osb = opool.tile([128, DIM], f32, tag="osb")
            for ds_ in range(2):
                ops_ = psB.tile([128, 2 * ELOC], f32, tag="b")
                for h in range(HLOC):
                    nc.tensor.matmul(ops_, yT[:, h, i * 128:(i + 1) * 128],
                                     tau_o[:, h, ds_ * 512:(ds_ + 1) * 512],
                                     start=(h == 0), stop=(h == HLOC - 1))
                nc.any.tensor_copy(out=osb[:, ds_ * 512:(ds_ + 1) * 512], in_=ops_)
            nc.sync.dma_start(out=outp[i * 128:(i + 1) * 128, :], in_=osb)

    if PHASE == 3:
        for i in range(NT):
            osb3 = opool.tile([128, 2 * HD], f32, tag="osb3")
            for h in range(HLOC):
                ps3 = psA.tile([128, 4, 128], f32, tag="a")
                nc.tensor.transpose(ps3[:, 0, :], yT[:, h, i * 128:(i + 1) * 128], ident)
                nc.any.tensor_copy(out=osb3[:, h * HD:(h + 1) * HD], in_=ps3[:, 0, :])
            nc.sync.dma_start(out=outp[i * 128:(i + 1) * 128, 0:2 * HD], in_=osb3)
    nc.compile()
    return nc


def _host_prep(inputs):
    x = np.ascontiguousarray(np.asarray(inputs["x"], np.float32))
    ve = np.ascontiguousarray(np.asarray(inputs["ve"], np.float32))
    lam = np.asarray(inputs["sa_lambdas"], np.float32)
    cos = np.asarray(inputs["cos"], np.float32)
    sin = np.asarray(inputs["sin"], np.float32)
    qkvo = np.asarray(inputs["qkvo_w"], np.float32)
    gw = np.asarray(inputs["gate_w"], np.float32)

    s_qkv = np.maximum(np.abs(qkvo[:3]).mean((1, 2), dtype=np.float32),
                       np.float32(1e-5)).astype(np.float32)
    s_o = np.float32(max(np.abs(qkvo[3]).mean(dtype=np.float32), np.float32(1e-5)))
    scal = np.empty((128, 8), np.float32)
    scal[:, 0:3] = s_qkv
    scal[:, 3] = s_o
    scal[:, 4:7] = np.float32(1.0) / s_qkv
    scal[:, 7] = np.float32(1.0) / s_o
    lam128 = np.ascontiguousarray(np.broadcast_to(lam, (128, 2)))

    c2 = np.concatenate([cos, cos], 1)            # [T,128]
    s2 = np.concatenate([sin, -sin], 1)           # [T,128]
    cos2 = np.ascontiguousarray(np.tile(c2, (1, HLOC)))   # [T,256]
    sin2 = np.ascontiguousarray(np.tile(s2, (1, HLOC)))

    in_maps = []
    for c in range(8):
        b, g = divmod(c, 4)
        rows = slice(g * ELOC, (g + 1) * ELOC)
        wq = np.concatenate([qkvo[s][rows].T for s in range(3)], axis=1)  # [1024,768]
        in_maps.append({
            "xb": x[b],
            "veb": np.ascontiguousarray(ve[b][:, rows]),
            "cos2": cos2,
            "sin2": sin2,
            "wqkvT": np.ascontiguousarray(wq),
            "woTq": np.ascontiguousarray(qkvo[3].T[rows]),
            "gwT": np.ascontiguousarray(gw[2 * g:2 * g + 2].T),
            "scal": scal,
            "lam": lam128,
        })
    return in_maps


def kernel(**inputs):
    from concourse.bass_utils import run_bass_kernel_spmd

    if "nc" not in _CACHE:
        _CACHE["nc"] = _build()
    nc = _CACHE["nc"]
    in_maps = _host_prep(inputs)
    res = run_bass_kernel_spmd(nc, in_maps, core_ids=list(range(8)))
    outs = [r["outp"] for r in res.results]
    out = np.empty((B, T, DIM), np.float32)
    for b in range(B):
        out[b] = outs[4 * b] + outs[4 * b + 1] + outs[4 * b + 2] + outs[4 * b + 3]
    return out


if __name__ == "__main__":
    import reference as R
    inputs = R.setup_inputs()
    out = kernel(**{k: np.asarray(v) for k, v in inputs.items()})
    print(out.shape, out.dtype)



# revision 8
# speedup vs baseline: 2.3399x; 2.3399x over previous
"""Trainium2 Bass kernel for nn_CausalSelfAttention (modded-nanogpt quantized attention).

Sharding: 8 cores = 2 batches x 4 head-groups (2 heads each). Each core
computes QKV for its 2 heads from x[b], runs causal attention + gating, and
produces a partial output projection (its 256 features of w_o); the host sums
the 4 fp16 partials per batch in fp32.

v2 design (fp16 / int8 everywhere):
 - host pre-quantizes x to int8 codes + per-token (pos, neg) scales; device
   reconstructs xq in fp16 (2 relu-scale ops + subtract), then DMA-XBAR
   transposes it to xqT [d, t] (no PE transposes anywhere).
 - ternary weights shipped as int8 {-1,0,1}, converted once to fp16; all
   matmuls fp16 (1 PE cycle/row vs 4 for fp32).
 - q/k chain: rms alpha folded into quant output scales (exact eps), rotary
   and two-branch int8 fake-quant done on [128, 2, 128] views with fp16
   magic-round (+1536-1536); q-chain on DVE, k-chain on Pool.
 - attention: S_T[tk,tq] = kT.T @ qT, E = exp(0.12*S - 8) in fp16 (the -8
   shift cancels in softmax and makes fp16 overflow impossible); y produced
   TRANSPOSED directly via yT += vaug.T @ E; denominator via ones-vector
   matmul into a [1,512] psum; gate sigmoid computed from the already-loaded
   Exp table; gate/den combined into one [1,512] factor, broadcast to
   [128,512] with a K=1 ones matmul, and multiplied into yT.
 - s_o folded into v (host), lam1*s_o folded into shipped ve, s_v*lam0*s_o
   shipped as a scalar; output projection accumulates 2 heads in PSUM and
   DMAs fp16 partials.
"""

import numpy as np

B, T, DIM, H, HD = 2, 2048, 1024, 8, 128
ATTN_SCALE = 0.12
F32_EPS = float(np.finfo(np.float32).eps)
EXP_SHIFT = -8.0          # exp(0.12*s - 8): |0.12*s| <= 15.6 so e^7.6 < fp16 max
MAGIC16 = 1536.0          # fp16 RNE round-to-int for |v| < 512
NT = T // 128             # 16 t-tiles
ND = DIM // 128           # 8 d-tiles
HLOC = 2                  # heads per core
ELOC = HLOC * HD          # 256 local features
NGRP = 4                  # 4 groups of 4 tiles; strip J = group

_CACHE = {}
DEBUG = False


def _build():
    import concourse.mybir as mybir
    import concourse.tile as tile
    from concourse import bacc
    from contextlib import ExitStack

    f32 = mybir.dt.float32
    f16 = mybir.dt.float16
    i8 = mybir.dt.int8
    A = mybir.AluOpType
    AF = mybir.ActivationFunctionType
    X = mybir.AxisListType.X

    nc = bacc.Bacc(trn_type="TRN2")

    # extra activation-bias constant (Bass pre-registers only 0.0/1.0)
    for _v in (EXP_SHIFT,):
        _t = nc.alloc_sbuf_tensor(f"const-float32-{_v}", [128, 1], f32)
        nc.gpsimd.memset(_t.ap(), _v)
        nc.const_aps.aps[(f32, _v)] = _t.ap()
    nc.all_engine_barrier()

    xb8 = nc.dram_tensor("xb8", [T, DIM], i8, kind="ExternalInput")
    sctok = nc.dram_tensor("sctok", [128, NT, 2], f32, kind="ExternalInput")
    veb = nc.dram_tensor("veb", [T, ELOC], f16, kind="ExternalInput")
    cosd = nc.dram_tensor("cosd", [T, HD], f16, kind="ExternalInput")
    sind = nc.dram_tensor("sind", [T, HD], f16, kind="ExternalInput")
    wqkv8 = nc.dram_tensor("wqkv8", [DIM, 3 * ELOC], i8, kind="ExternalInput")
    wo8 = nc.dram_tensor("wo8", [ELOC, DIM], i8, kind="ExternalInput")
    gwT = nc.dram_tensor("gwT", [12, HLOC], f16, kind="ExternalInput")
    # scal cols 0-3: s^2/HD per (scol,h); cols 4-7: s per (scol,h)
    scal = nc.dram_tensor("scal", [128, 8], f32, kind="ExternalInput")
    lam = nc.dram_tensor("lam", [128, 2], f32, kind="ExternalInput")
    outp = nc.dram_tensor("outp", [T, DIM], f16, kind="ExternalOutput")
    if DEBUG:
        dbg_xq = nc.dram_tensor("dbg_xq", [T, DIM], f16, kind="ExternalOutput")
        dbg_qq = nc.dram_tensor("dbg_qq", [T, 2 * ELOC], f16, kind="ExternalOutput")
        dbg_v = nc.dram_tensor("dbg_v", [T, ELOC], f16, kind="ExternalOutput")
        dbg_g = nc.dram_tensor("dbg_g", [HLOC, T], f16, kind="ExternalOutput")
        dbg_y = nc.dram_tensor("dbg_y", [128, HLOC, T], f16, kind="ExternalOutput")

    with tile.TileContext(nc) as tc, ExitStack() as ctx:
        singles = ctx.enter_context(tc.tile_pool(name="singles", bufs=1))
        xpool = ctx.enter_context(tc.tile_pool(name="xpool", bufs=2))
        cpool = ctx.enter_context(tc.tile_pool(name="cpool", bufs=2))
        spool = ctx.enter_context(tc.tile_pool(name="spool", bufs=2))
        epool = ctx.enter_context(tc.tile_pool(name="epool", bufs=4))
        opool = ctx.enter_context(tc.tile_pool(name="opool", bufs=2))
        psQ = ctx.enter_context(tc.tile_pool(name="psQ", bufs=1, space="PSUM"))
        psS = ctx.enter_context(tc.tile_pool(name="psS", bufs=2, space="PSUM"))
        psY = ctx.enter_context(tc.tile_pool(name="psY", bufs=2, space="PSUM"))
        psD = ctx.enter_context(tc.tile_pool(name="psD", bufs=2, space="PSUM"))

        def ts(out, in0, s1, s2=None, op0=A.mult, op1=None, eng=None):
            e = eng if eng is not None else nc.vector
            kw = {}
            if op1 is not None:
                kw["op1"] = op1
            e.tensor_scalar(out=out, in0=in0, scalar1=s1, scalar2=s2, op0=op0, **kw)

        # ---------------- small persistent inputs ----------------
        scal_sb = singles.tile([128, 8], f32)
        nc.sync.dma_start(out=scal_sb, in_=scal[:, :])
        lam_sb = singles.tile([128, 2], f32)
        nc.sync.dma_start(out=lam_sb, in_=lam[:, :])
        gw_sb = singles.tile([12, HLOC], f16)
        nc.sync.dma_start(out=gw_sb, in_=gwT[:, :])
        sct = singles.tile([128, NT, 2], f32)
        nc.sync.dma_start(out=sct, in_=sctok[:, :, :])
        cosb = singles.tile([128, NT, HD], f16)
        nc.sync.dma_start(out=cosb, in_=cosd.rearrange("(n p) d -> p n d", p=128))
        sinb = singles.tile([128, NT, HD], f16)
        nc.sync.dma_start(out=sinb, in_=sind.rearrange("(n p) d -> p n d", p=128))

        ones1 = singles.tile([1, 128], f16)
        nc.gpsimd.memset(ones1, 1.0)
        onesC = singles.tile([128, 1], f16)
        nc.gpsimd.memset(onesC, 1.0)

        # ---------------- weights: int8 -> fp16 ----------------
        tau8 = singles.tile([128, ND, 3 * ELOC], i8)
        nc.sync.dma_start(out=tau8, in_=wqkv8.rearrange("(n p) e -> p n e", p=128))
        tau = singles.tile([128, ND, 3 * ELOC], f16)
        nc.vector.tensor_copy(out=tau[:, 0:3, :], in_=tau8[:, 0:3, :])
        nc.gpsimd.tensor_copy(out=tau[:, 3:6, :], in_=tau8[:, 3:6, :])
        nc.scalar.copy(out=tau[:, 6:8, :], in_=tau8[:, 6:8, :])
        tau_o8 = singles.tile([128, HLOC, DIM], i8)
        nc.sync.dma_start(out=tau_o8, in_=wo8.rearrange("(h p) d -> p h d", p=128))
        tau_o = singles.tile([128, HLOC, DIM], f16)
        nc.vector.tensor_copy(out=tau_o[:, 0, :], in_=tau_o8[:, 0, :])
        nc.gpsimd.tensor_copy(out=tau_o[:, 1, :], in_=tau_o8[:, 1, :])

        # ---------------- persistent activations ----------------
        # [dp, tile, h, t] layouts so per-tile writes are contiguous
        qT = singles.tile([128, NT, HLOC, 128], f16)
        kT = singles.tile([128, NT, HLOC, 128], f16)
        vaug = singles.tile([128, NT, HLOC, 128], f16)
        yT = singles.tile([128, HLOC, NGRP, 512], f16)
        gateZ0 = singles.tile([1, T], f16)
        gateZ1 = singles.tile([1, T], f16)
        gateZ = [gateZ0, gateZ1]

        def qkv_tile(i):
            k8 = xpool.tile([128, DIM], i8, tag="k8")
            nc.sync.dma_start(out=k8, in_=xb8[i * 128:(i + 1) * 128, :])
            # reconstruct xq fp16: pos on ACT, neg on DVE, sub on Pool
            pos = xpool.tile([128, DIM], f16, tag="pos")
            nc.scalar.activation(pos, k8, AF.Relu, scale=sct[:, i, 0:1])
            neg = xpool.tile([128, DIM], f16, tag="neg")
            ts(neg, k8, 0.0, sct[:, i, 1:2], A.min, A.mult, eng=nc.vector)
            xq = xpool.tile([128, DIM], f16, tag="xq")
            nc.gpsimd.tensor_tensor(out=xq, in0=pos, in1=neg, op=A.subtract)
            if DEBUG:
                nc.sync.dma_start(out=dbg_xq[i * 128:(i + 1) * 128, :], in_=xq)
            xqT = xpool.tile([128, ND, 128], f16, tag="xqT")
            nc.sync.dma_start_transpose(out=xqT, in_=xq)

            # gate logits (transposed), one partition-0 row per head
            for h in range(HLOC):
                gps = psS.tile([1, 128], f32, tag="s")
                nc.tensor.matmul(gps, gw_sb[:, h:h + 1], xqT[0:12, 0, :],
                                 start=True, stop=True)
                nc.scalar.copy(out=gateZ[h][:, i * 128:(i + 1) * 128], in_=gps)

            # QKV matmuls
            qkv_ps = psQ.tile([128, 3 * ELOC], f32, tag="qkv")
            for d in range(ND):
                nc.tensor.matmul(qkv_ps[:, 0:512], xqT[:, d, :], tau[:, d, 0:512],
                                 start=(d == 0), stop=(d == ND - 1))
                nc.tensor.matmul(qkv_ps[:, 512:768], xqT[:, d, :],
                                 tau[:, d, 512:768],
                                 start=(d == 0), stop=(d == ND - 1))

            # v mix into vaug (ve pre-scaled by lam1*s_o on host)
            vet = cpool.tile([128, ELOC], f16, tag="vet")
            nc.sync.dma_start(out=vet, in_=veb[i * 128:(i + 1) * 128, :])
            nc.vector.scalar_tensor_tensor(
                out=vaug[:, i, :, :], in0=qkv_ps[:, 512:768].rearrange(
                    "p (h d) -> p h d", h=HLOC),
                scalar=lam_sb[:, 0:1],
                in1=vet.rearrange("p (h d) -> p h d", h=HLOC),
                op0=A.mult, op1=A.add)
            if DEBUG:
                nc.sync.dma_start(out=dbg_v[i * 128:(i + 1) * 128, :],
                                  in_=vaug[:, i, :, :].rearrange("p h d -> p (h d)"))

            # ---- sum of squares -> alpha (rms fold, exact eps) ----
            junk = cpool.tile([128, 512], f32, tag="junk")
            nc.scalar.activation(junk, qkv_ps[:, 0:512], AF.Square)
            sq4 = cpool.tile([128, 4, 1], f32, tag="sq4")
            nc.vector.tensor_reduce(out=sq4, in_=junk.rearrange(
                "p (a b) -> p a b", a=4), axis=X, op=A.add)
            nc.vector.tensor_tensor(
                out=sq4, in0=sq4,
                in1=scal_sb[:, 0:4].rearrange("p (a b) -> p a b", b=1), op=A.mult)
            ts(sq4, sq4, F32_EPS, None, A.add)
            nc.scalar.sqrt(sq4, sq4)
            rc4 = cpool.tile([128, 4, 1], f32, tag="rc4")
            nc.vector.reciprocal(out=rc4, in_=sq4)
            al4 = cpool.tile([128, 4, 1], f32, tag="al4")
            nc.vector.tensor_tensor(
                out=al4, in0=rc4,
                in1=scal_sb[:, 4:8].rearrange("p (a b) -> p a b", b=1), op=A.mult)

            # ---- natural fp16 copy + rotary (q on DVE, k on Pool) ----
            nat = cpool.tile([128, 2, 2, 128], f16, tag="nat")  # [p, scol, h, d]
            rot = cpool.tile([128, 2, 2, 128], f16, tag="rot")
            t2 = cpool.tile([128, 2, 2, 128], f16, tag="t2")
            nc.vector.tensor_copy(out=nat[:, 0, :, :],
                                  in_=qkv_ps[:, 0:256].rearrange(
                                      "p (h d) -> p h d", h=HLOC))
            nc.scalar.copy(out=nat[:, 1, :, :],
                           in_=qkv_ps[:, 256:512].rearrange(
                               "p (h d) -> p h d", h=HLOC))
            for s, eng in ((0, nc.vector), (1, nc.gpsimd)):
                cb = cosb[:, i:i + 1, :].to_broadcast([128, HLOC, HD])
                eng.tensor_tensor(out=rot[:, s], in0=nat[:, s], in1=cb, op=A.mult)
                s1 = sinb[:, i:i + 1, 0:64].to_broadcast([128, HLOC, 64])
                s2 = sinb[:, i:i + 1, 64:128].to_broadcast([128, HLOC, 64])
                eng.tensor_tensor(out=t2[:, s, :, 0:64], in0=nat[:, s, :, 64:128],
                                  in1=s1, op=A.mult)
                eng.tensor_tensor(out=t2[:, s, :, 64:128], in0=nat[:, s, :, 0:64],
                                  in1=s2, op=A.mult)
                eng.tensor_tensor(out=rot[:, s], in0=rot[:, s], in1=t2[:, s],
                                  op=A.add)

            # ---- per-(scol,head) quant scales ----
            mx8 = cpool.tile([128, 8, 1], f32, tag="mx8")  # 0:4 max, 4:8 min
            nc.vector.tensor_reduce(out=mx8[:, 0:4], in_=rot.rearrange(
                "p a h d -> p (a h) d"), axis=X, op=A.max)
            nc.vector.tensor_reduce(out=mx8[:, 4:8], in_=rot.rearrange(
                "p a h d -> p (a h) d"), axis=X, op=A.min)
            ts(mx8[:, 0:4], mx8[:, 0:4], 1e-5, None, A.max)
            ts(mx8[:, 4:8], mx8[:, 4:8], -1e-5, None, A.min)
            rcp8 = cpool.tile([128, 8, 1], f32, tag="rcp8")
            nc.vector.reciprocal(out=rcp8, in_=mx8)
            msc = cpool.tile([128, 8, 1], f16, tag="msc")   # 127/max, 127/min
            ts(msc, rcp8, 127.0)
            qsc = cpool.tile([128, 8, 1], f16, tag="qsc")   # max*al/127, min*al/127
            for half in range(2):
                nc.vector.scalar_tensor_tensor(
                    out=qsc[:, 4 * half:4 * half + 4], in0=mx8[:, 4 * half:4 * half + 4],
                    scalar=1.0 / 127.0, in1=al4, op0=A.mult, op1=A.mult)

            # ---- two-branch fake-quant application ----
            qq = cpool.tile([128, 2, 2, 128], f16, tag="qq")
            tb = cpool.tile([128, 2, 2, 128], f16, tag="tb")
            for s, eng in ((0, nc.vector), (1, nc.gpsimd)):
                pslc = msc[:, 2 * s:2 * s + 2].to_broadcast([128, HLOC, 128])
                nslc = msc[:, 4 + 2 * s:6 + 2 * s].to_broadcast([128, HLOC, 128])
                pq = qsc[:, 2 * s:2 * s + 2].to_broadcast([128, HLOC, 128])
                nq = qsc[:, 4 + 2 * s:6 + 2 * s].to_broadcast([128, HLOC, 128])
                if eng is nc.vector:  # STT is DVE-only
                    eng.scalar_tensor_tensor(out=qq[:, s], in0=rot[:, s],
                                             scalar=0.0, in1=pslc,
                                             op0=A.max, op1=A.mult)
                    eng.scalar_tensor_tensor(out=tb[:, s], in0=rot[:, s],
                                             scalar=0.0, in1=nslc,
                                             op0=A.min, op1=A.mult)
                else:
                    ts(qq[:, s], rot[:, s], 0.0, None, A.max, eng=eng)
                    eng.tensor_tensor(out=qq[:, s], in0=qq[:, s], in1=pslc,
                                      op=A.mult)
                    ts(tb[:, s], rot[:, s], 0.0, None, A.min, eng=eng)
                    eng.tensor_tensor(out=tb[:, s], in0=tb[:, s], in1=nslc,
                                      op=A.mult)
                ts(qq[:, s], qq[:, s], MAGIC16, MAGIC16, A.add, A.subtract, eng=eng)
                eng.tensor_tensor(out=qq[:, s], in0=qq[:, s], in1=pq, op=A.mult)
                ts(tb[:, s], tb[:, s], MAGIC16, MAGIC16, A.add, A.subtract, eng=eng)
                eng.tensor_tensor(out=tb[:, s], in0=tb[:, s], in1=nq, op=A.mult)
                eng.tensor_tensor(out=qq[:, s], in0=qq[:, s], in1=tb[:, s], op=A.add)
            if DEBUG:
                nc.sync.dma_start(out=dbg_qq[i * 128:(i + 1) * 128, :],
                                  in_=qq.rearrange("p a h d -> p (a h d)"))

            qf = qq.rearrange("p a h d -> p (a h d)")
            nc.sync.dma_start_transpose(out=qT[:, i, :, :], in_=qf[:, 0:256])
            nc.sync.dma_start_transpose(out=kT[:, i, :, :], in_=qf[:, 256:512])

        def attn_strip(J, h):
            yps = psY.tile([128, 512], f32, tag="y")
            dps = psD.tile([1, 512], f32, tag="den")
            nblk = 4 * J + 4
            for i in range(nblk):
                st = psS.tile([128, 512], f32, tag="s")
                nc.tensor.matmul(st, kT[:, i, h, :], qT[:, 4 * J:4 * J + 4, h, :],
                                 start=True, stop=True)
                lo = max(0, 128 * (i - 4 * J))
                E = epool.tile([128, 512], f16, tag="E")
                nc.scalar.activation(E[:, lo:512], st[:, lo:512], AF.Exp,
                                     scale=ATTN_SCALE, bias=EXP_SHIFT)
                if i >= 4 * J:
                    nc.gpsimd.affine_select(
                        out=E[:, lo:lo + 128], in_=E[:, lo:lo + 128],
                        compare_op=A.is_ge, fill=0.0, base=0,
                        pattern=[[1, 128]], channel_multiplier=-1)
                nc.tensor.matmul(yps[:, lo:512], vaug[:, i, h, :], E[:, lo:512],
                                 start=(i == 0), stop=(i == nblk - 1))
                nc.tensor.matmul(dps[:, lo:512], onesC, E[:, lo:512],
                                 start=(i == 0), stop=(i == nblk - 1))
            # gate sigmoid via Exp table: g = 1/(1+exp(-z)); fac = g/den
            eg = spool.tile([1, 512], f32, tag="eg")
            nc.scalar.activation(eg, gateZ[h][:, J * 512:(J + 1) * 512],
                                 AF.Exp, scale=-1.0)
            ts(eg, eg, 1.0, None, A.add)
            nc.vector.tensor_tensor(out=eg, in0=eg, in1=dps, op=A.mult)
            fac32 = spool.tile([1, 512], f32, tag="fac32")
            nc.vector.reciprocal(out=fac32, in_=eg)
            fac16 = spool.tile([1, 512], f16, tag="fac16")
            nc.vector.tensor_copy(out=fac16, in_=fac32)
            fps = psS.tile([128, 512], f32, tag="s")
            nc.tensor.matmul(fps, ones1, fac16, start=True, stop=True)
            facb = spool.tile([128, 512], f16, tag="facb")
            nc.scalar.copy(out=facb, in_=fps)
            nc.vector.tensor_tensor(out=yT[:, h, J, :], in0=yps, in1=facb,
                                    op=A.mult)
            if DEBUG:
                nc.sync.dma_start(out=dbg_g[h:h + 1, J * 512:(J + 1) * 512],
                                  in_=fac16)  # noqa
                nc.sync.dma_start(out=dbg_y[:, h, J * 512:(J + 1) * 512],
                                  in_=yT[:, h, J, :])

        def out_tile(i):
            J, jj = divmod(i, 4)
            osb = opool.tile([128, DIM], f16, tag="osb")
            for half in range(2):
                ops_ = psY.tile([128, 512], f32, tag="y")
                for h in range(HLOC):
                    nc.tensor.matmul(ops_, yT[:, h, J, jj * 128:(jj + 1) * 128],
                                     tau_o[:, h, half * 512:(half + 1) * 512],
                                     start=(h == 0), stop=(h == HLOC - 1))
                if half == 0:
                    nc.vector.tensor_copy(out=osb[:, 0:512], in_=ops_)
                else:
                    nc.scalar.copy(out=osb[:, 512:1024], in_=ops_)
            nc.sync.dma_start(out=outp[i * 128:(i + 1) * 128, :], in_=osb)

        for grp in range(NGRP):
            for i in range(4 * grp, 4 * grp + 4):
                qkv_tile(i)
            for h in range(HLOC):
                attn_strip(grp, h)
            for i in range(4 * grp, 4 * grp + 4):
                out_tile(i)

    nc.compile()
    return nc


def _host_prep(inputs):
    x = np.asarray(inputs["x"], np.float32)
    ve = np.asarray(inputs["ve"], np.float32)
    lam = np.asarray(inputs["sa_lambdas"], np.float32)
    cos = np.asarray(inputs["cos"], np.float32)
    sin = np.asarray(inputs["sin"], np.float32)
    qkvo = np.asarray(inputs["qkvo_w"], np.float32)
    gw = np.asarray(inputs["gate_w"], np.float32)

    # weight ternary quantization (global scales), exact fp32 mirror of ref
    s_qkv = np.maximum(np.abs(qkvo[:3]).mean((1, 2), dtype=np.float32),
                       np.float32(1e-5)).astype(np.float32)
    s_o = np.float32(max(np.abs(qkvo[3]).mean(dtype=np.float32), np.float32(1e-5)))
    tern_qkv = np.clip(np.round(qkvo[:3] / s_qkv[:, None, None]), -1, 1
                       ).astype(np.int8)
    tern_o = np.clip(np.round(qkvo[3] / s_o), -1, 1).astype(np.int8)

    # x int8 fake-quant codes + per-token scales, exact fp32 mirror of ref
    k8b, scb = [], []
    for b in range(B):
        xb = x[b]
        xpmax = np.maximum(xb.max(-1, keepdims=True), np.float32(1e-5))
        xnmin = np.minimum(xb.min(-1, keepdims=True), np.float32(-1e-5))
        rp = np.round((xb / xpmax) * np.float32(127.0))
        rn = np.round((xb / xnmin) * np.float32(127.0))
        k = np.where(xb >= 0, rp, -rn).astype(np.int8)
        sc = np.concatenate([xpmax / np.float32(127.0),
                             xnmin / np.float32(127.0)], 1)  # [T, 2]
        k8b.append(k)
        # [128, NT, 2]: partition-major pack
        scb.append(np.ascontiguousarray(
            sc.reshape(NT, 128, 2).transpose(1, 0, 2)))

    c2 = np.concatenate([cos, cos], 1).astype(np.float16)   # [T,128]
    s2 = np.concatenate([sin, -sin], 1).astype(np.float16)

    scal = np.empty((128, 8), np.float32)
    scal[:, 0] = scal[:, 1] = s_qkv[0] * s_qkv[0] / np.float32(HD)
    scal[:, 2] = scal[:, 3] = s_qkv[1] * s_qkv[1] / np.float32(HD)
    scal[:, 4] = scal[:, 5] = s_qkv[0]
    scal[:, 6] = scal[:, 7] = s_qkv[1]
    lam128 = np.empty((128, 2), np.float32)
    lam128[:, 0] = lam[0] * s_qkv[2] * s_o
    lam128[:, 1] = 0.0

    in_maps = []
    for c in range(8):
        b, g = divmod(c, 4)
        rows = slice(g * ELOC, (g + 1) * ELOC)
        wq = np.concatenate([tern_qkv[s][rows].T for s in range(3)], axis=1)
        in_maps.append({
            "xb8": k8b[b],
            "sctok": scb[b],
            "veb": (ve[b][:, rows] * (lam[1] * s_o)).astype(np.float16),
            "cosd": c2,
            "sind": s2,
            "wqkv8": np.ascontiguousarray(wq),
            "wo8": np.ascontiguousarray(tern_o.T[rows]),
            "gwT": np.ascontiguousarray(gw[2 * g:2 * g + 2].T).astype(np.float16),
            "scal": scal,
            "lam": lam128,
        })
    return in_maps


def kernel(**inputs):
    from concourse.bass_utils import run_bass_kernel_spmd

    if "nc" not in _CACHE:
        _CACHE["nc"] = _build()
    nc = _CACHE["nc"]
    in_maps = _host_prep(inputs)
    res = run_bass_kernel_spmd(nc, in_maps, core_ids=list(range(8)))
    outs = [r["outp"].astype(np.float32) for r in res.results]
    out = np.empty((B, T, DIM), np.float32)
    for b in range(B):
        out[b] = outs[4 * b] + outs[4 * b + 1] + outs[4 * b + 2] + outs[4 * b + 3]
    return out


if __name__ == "__main__":
    import reference as R
    inputs = R.setup_inputs()
    out = kernel(**{k: np.asarray(v) for k, v in inputs.items()})
    print(out.shape, out.dtype)


# revision 12
# speedup vs baseline: 3.8265x; 1.6353x over previous
"""Trainium2 Bass kernel for nn_CausalSelfAttention (modded-nanogpt quantized attention).

Sharding: 8 cores = 2 batches x 4 head-groups (2 heads each). Each core
computes QKV for its 2 heads from x[b], runs causal attention + gating, and
produces a partial output projection (its 256 features of w_o); the host sums
the 4 fp16 partials per batch in fp32.

v2 design (fp16 / int8 everywhere):
 - host pre-quantizes x to int8 codes + per-token (pos, neg) scales; device
   reconstructs xq in fp16 (2 relu-scale ops + subtract), then DMA-XBAR
   transposes it to xqT [d, t] (no PE transposes anywhere).
 - ternary weights shipped as int8 {-1,0,1}, converted once to fp16; all
   matmuls fp16 (1 PE cycle/row vs 4 for fp32).
 - q/k chain: rms alpha folded into quant output scales (exact eps), rotary
   and two-branch int8 fake-quant done on [128, 2, 128] views with fp16
   magic-round (+1536-1536); q-chain on DVE, k-chain on Pool.
 - attention: S_T[tk,tq] = kT.T @ qT, E = exp(0.12*S - 8) in fp16 (the -8
   shift cancels in softmax and makes fp16 overflow impossible); y produced
   TRANSPOSED directly via yT += vaug.T @ E; denominator via ones-vector
   matmul into a [1,512] psum; gate sigmoid computed from the already-loaded
   Exp table; gate/den combined into one [1,512] factor, broadcast to
   [128,512] with a K=1 ones matmul, and multiplied into yT.
 - s_o folded into v (host), lam1*s_o folded into shipped ve, s_v*lam0*s_o
   shipped as a scalar; output projection accumulates 2 heads in PSUM and
   DMAs fp16 partials.
"""

import numpy as np

B, T, DIM, H, HD = 2, 2048, 1024, 8, 128
ATTN_SCALE = 0.12
F32_EPS = float(np.finfo(np.float32).eps)
EXP_SHIFT = -8.0          # exp(0.12*s - 8): |0.12*s| <= 15.6 so e^7.6 < fp16 max
MAGIC16 = 1536.0          # fp16 RNE round-to-int for |v| < 512
NT = T // 128             # 16 t-tiles
ND = DIM // 128           # 8 d-tiles
HLOC = 2                  # heads per core
ELOC = HLOC * HD          # 256 local features
NGRP = 4                  # 4 groups of 4 tiles; strip J = group
USE_RS = True             # device-side ReduceScatter of output partials

_CACHE = {}
DEBUG = False


def _build():
    import concourse.mybir as mybir
    import concourse.tile as tile
    from concourse import bacc
    from contextlib import ExitStack

    f32 = mybir.dt.float32
    f16 = mybir.dt.float16
    i8 = mybir.dt.int8
    A = mybir.AluOpType
    AF = mybir.ActivationFunctionType
    X = mybir.AxisListType.X

    nc = bacc.Bacc(trn_type="TRN2")

    # extra activation-bias constant (Bass pre-registers only 0.0/1.0)
    for _v in (EXP_SHIFT,):
        _t = nc.alloc_sbuf_tensor(f"const-float32-{_v}", [128, 1], f32)
        nc.gpsimd.memset(_t.ap(), _v)
        nc.const_aps.aps[(f32, _v)] = _t.ap()
    nc.all_engine_barrier()

    xb8 = nc.dram_tensor("xb8", [T, DIM], i8, kind="ExternalInput")
    sctok = nc.dram_tensor("sctok", [128, NT, 2], f32, kind="ExternalInput")
    veb = nc.dram_tensor("veb", [T, ELOC], f16, kind="ExternalInput")
    cosd = nc.dram_tensor("cosd", [T, HD], f16, kind="ExternalInput")
    sind = nc.dram_tensor("sind", [T, HD], f16, kind="ExternalInput")
    wqkv8 = nc.dram_tensor("wqkv8", [DIM, 3 * ELOC], i8, kind="ExternalInput")
    wo8 = nc.dram_tensor("wo8", [ELOC, DIM], i8, kind="ExternalInput")
    gwT = nc.dram_tensor("gwT", [12, HLOC], f16, kind="ExternalInput")
    # scal cols 0-3: s^2/HD per (scol,h); cols 4-7: s per (scol,h)
    scal = nc.dram_tensor("scal", [128, 8], f32, kind="ExternalInput")
    lam = nc.dram_tensor("lam", [128, 2], f32, kind="ExternalInput")
    outp = nc.dram_tensor("outp", [T // 4 if USE_RS else T, DIM], f16,
                          kind="ExternalOutput")
    if DEBUG:
        dbg_xq = nc.dram_tensor("dbg_xq", [T, DIM], f16, kind="ExternalOutput")
        dbg_qq = nc.dram_tensor("dbg_qq", [T, 2 * ELOC], f16, kind="ExternalOutput")
        dbg_v = nc.dram_tensor("dbg_v", [T, ELOC], f16, kind="ExternalOutput")
        dbg_g = nc.dram_tensor("dbg_g", [HLOC, T], f16, kind="ExternalOutput")
        dbg_y = nc.dram_tensor("dbg_y", [128, HLOC, T], f16, kind="ExternalOutput")

    with tile.TileContext(nc) as tc, ExitStack() as ctx:
        singles = ctx.enter_context(tc.tile_pool(name="singles", bufs=1))
        xpool = ctx.enter_context(tc.tile_pool(name="xpool", bufs=2))
        cpool = ctx.enter_context(tc.tile_pool(name="cpool", bufs=2))
        spool = ctx.enter_context(tc.tile_pool(name="spool", bufs=2))
        epool = ctx.enter_context(tc.tile_pool(name="epool", bufs=4))
        opool = ctx.enter_context(tc.tile_pool(name="opool", bufs=2))
        psQ = ctx.enter_context(tc.tile_pool(name="psQ", bufs=1, space="PSUM"))
        psS = ctx.enter_context(tc.tile_pool(name="psS", bufs=2, space="PSUM"))
        psY = ctx.enter_context(tc.tile_pool(name="psY", bufs=2, space="PSUM"))
        psD = ctx.enter_context(tc.tile_pool(name="psD", bufs=2, space="PSUM"))
        dpool = ctx.enter_context(tc.tile_pool(name="dpool", bufs=1, space="DRAM"))

        def ts(out, in0, s1, s2=None, op0=A.mult, op1=None, eng=None):
            e = eng if eng is not None else nc.vector
            kw = {}
            if op1 is not None:
                kw["op1"] = op1
            e.tensor_scalar(out=out, in0=in0, scalar1=s1, scalar2=s2, op0=op0, **kw)

        # ---------------- small persistent inputs ----------------
        scal_sb = singles.tile([128, 8], f32)
        nc.sync.dma_start(out=scal_sb, in_=scal[:, :])
        lam_sb = singles.tile([128, 2], f32)
        nc.sync.dma_start(out=lam_sb, in_=lam[:, :])
        gw_sb = singles.tile([12, HLOC], f16)
        nc.sync.dma_start(out=gw_sb, in_=gwT[:, :])
        sct = singles.tile([128, NT, 2], f32)
        nc.sync.dma_start(out=sct, in_=sctok[:, :, :])
        cosb = singles.tile([128, NT, HD], f16)
        nc.sync.dma_start(out=cosb, in_=cosd.rearrange("(n p) d -> p n d", p=128))
        sinb = singles.tile([128, NT, HD], f16)
        nc.sync.dma_start(out=sinb, in_=sind.rearrange("(n p) d -> p n d", p=128))

        ones1 = singles.tile([1, 128], f16)
        nc.gpsimd.memset(ones1, 1.0)
        onesC = singles.tile([128, 1], f16)
        nc.gpsimd.memset(onesC, 1.0)

        # ---------------- weights: int8 -> fp16 ----------------
        tau8 = singles.tile([128, ND, 3 * ELOC], i8)
        nc.sync.dma_start(out=tau8, in_=wqkv8.rearrange("(n p) e -> p n e", p=128))
        tau = singles.tile([128, ND, 3 * ELOC], f16)
        nc.vector.tensor_copy(out=tau[:, 0:3, :], in_=tau8[:, 0:3, :])
        nc.gpsimd.tensor_copy(out=tau[:, 3:6, :], in_=tau8[:, 3:6, :])
        nc.scalar.copy(out=tau[:, 6:8, :], in_=tau8[:, 6:8, :])
        tau_o8 = singles.tile([128, HLOC, DIM], i8)
        nc.sync.dma_start(out=tau_o8, in_=wo8.rearrange("(h p) d -> p h d", p=128))
        tau_o = singles.tile([128, HLOC, DIM], f16)
        nc.vector.tensor_copy(out=tau_o[:, 0, :], in_=tau_o8[:, 0, :])
        nc.gpsimd.tensor_copy(out=tau_o[:, 1, :], in_=tau_o8[:, 1, :])

        # ---------------- persistent activations ----------------
        # [dp, tile, h, t] layouts so per-tile writes are contiguous
        qT = singles.tile([128, NT, HLOC, 128], f16)
        kT = singles.tile([128, NT, HLOC, 128], f16)
        vaug = singles.tile([128, NT, HLOC, 128], f16)
        yT = singles.tile([128, HLOC, NGRP, 512], f16)
        gateZ0 = singles.tile([1, T], f16)
        gateZ1 = singles.tile([1, T], f16)
        gateZ = [gateZ0, gateZ1]
        part = dpool.tile([T, DIM], f16, name="part") if USE_RS else None
        rs_out = dpool.tile([T // 4, DIM], f16, name="rs_out") if USE_RS else None

        def qkv_tile(i):
            k8 = xpool.tile([128, DIM], i8, tag="k8")
            nc.sync.dma_start(out=k8, in_=xb8[i * 128:(i + 1) * 128, :])
            # reconstruct xq fp16: pos on ACT, neg on DVE, sub on Pool
            pos = xpool.tile([128, DIM], f16, tag="pos")
            nc.scalar.activation(pos, k8, AF.Relu, scale=sct[:, i, 0:1])
            neg = xpool.tile([128, DIM], f16, tag="neg")
            ts(neg, k8, 0.0, sct[:, i, 1:2], A.min, A.mult, eng=nc.vector)
            xq = xpool.tile([128, DIM], f16, tag="xq")
            nc.gpsimd.tensor_tensor(out=xq, in0=pos, in1=neg, op=A.subtract)
            if DEBUG:
                nc.sync.dma_start(out=dbg_xq[i * 128:(i + 1) * 128, :], in_=xq)
            xqT = xpool.tile([128, ND, 128], f16, tag="xqT")
            nc.sync.dma_start_transpose(out=xqT, in_=xq)

            # gate logits (transposed), one partition-0 row per head
            for h in range(HLOC):
                gps = psS.tile([1, 128], f32, tag="s")
                nc.tensor.matmul(gps, gw_sb[:, h:h + 1], xqT[0:12, 0, :],
                                 start=True, stop=True)
                nc.scalar.copy(out=gateZ[h][:, i * 128:(i + 1) * 128], in_=gps)

            # QKV matmuls
            qkv_ps = psQ.tile([128, 3 * ELOC], f32, tag="qkv")
            for d in range(ND):
                nc.tensor.matmul(qkv_ps[:, 0:512], xqT[:, d, :], tau[:, d, 0:512],
                                 start=(d == 0), stop=(d == ND - 1))
                nc.tensor.matmul(qkv_ps[:, 512:768], xqT[:, d, :],
                                 tau[:, d, 512:768],
                                 start=(d == 0), stop=(d == ND - 1))

            # v mix into vaug (ve pre-scaled by lam1*s_o on host)
            vet = cpool.tile([128, ELOC], f16, tag="vet")
            nc.sync.dma_start(out=vet, in_=veb[i * 128:(i + 1) * 128, :])
            nc.vector.scalar_tensor_tensor(
                out=vaug[:, i, :, :], in0=qkv_ps[:, 512:768].rearrange(
                    "p (h d) -> p h d", h=HLOC),
                scalar=lam_sb[:, 0:1],
                in1=vet.rearrange("p (h d) -> p h d", h=HLOC),
                op0=A.mult, op1=A.add)
            if DEBUG:
                nc.sync.dma_start(out=dbg_v[i * 128:(i + 1) * 128, :],
                                  in_=vaug[:, i, :, :].rearrange("p h d -> p (h d)"))

            # ---- sum of squares -> alpha (rms fold, exact eps) ----
            junk = cpool.tile([128, 512], f32, tag="junk")
            nc.scalar.activation(junk, qkv_ps[:, 0:512], AF.Square)
            sq4 = cpool.tile([128, 4, 1], f32, tag="sq4")
            nc.vector.tensor_reduce(out=sq4, in_=junk.rearrange(
                "p (a b) -> p a b", a=4), axis=X, op=A.add)
            nc.vector.tensor_tensor(
                out=sq4, in0=sq4,
                in1=scal_sb[:, 0:4].rearrange("p (a b) -> p a b", b=1), op=A.mult)
            ts(sq4, sq4, F32_EPS, None, A.add)
            nc.scalar.sqrt(sq4, sq4)
            rc4 = cpool.tile([128, 4, 1], f32, tag="rc4")
            nc.vector.reciprocal(out=rc4, in_=sq4)
            al4 = cpool.tile([128, 4, 1], f32, tag="al4")
            nc.vector.tensor_tensor(
                out=al4, in0=rc4,
                in1=scal_sb[:, 4:8].rearrange("p (a b) -> p a b", b=1), op=A.mult)

            # ---- natural fp16 copy + rotary (q on DVE, k on Pool) ----
            nat = cpool.tile([128, 2, 2, 128], f16, tag="nat")  # [p, scol, h, d]
            rot = cpool.tile([128, 2, 2, 128], f16, tag="rot")
            t2 = cpool.tile([128, 2, 2, 128], f16, tag="t2")
            nc.vector.tensor_copy(out=nat[:, 0, :, :],
                                  in_=qkv_ps[:, 0:256].rearrange(
                                      "p (h d) -> p h d", h=HLOC))
            nc.scalar.copy(out=nat[:, 1, :, :],
                           in_=qkv_ps[:, 256:512].rearrange(
                               "p (h d) -> p h d", h=HLOC))
            for s, eng in ((0, nc.vector), (1, nc.gpsimd)):
                cb = cosb[:, i:i + 1, :].to_broadcast([128, HLOC, HD])
                eng.tensor_tensor(out=rot[:, s], in0=nat[:, s], in1=cb, op=A.mult)
                s1 = sinb[:, i:i + 1, 0:64].to_broadcast([128, HLOC, 64])
                s2 = sinb[:, i:i + 1, 64:128].to_broadcast([128, HLOC, 64])
                eng.tensor_tensor(out=t2[:, s, :, 0:64], in0=nat[:, s, :, 64:128],
                                  in1=s1, op=A.mult)
                eng.tensor_tensor(out=t2[:, s, :, 64:128], in0=nat[:, s, :, 0:64],
                                  in1=s2, op=A.mult)
                eng.tensor_tensor(out=rot[:, s], in0=rot[:, s], in1=t2[:, s],
                                  op=A.add)

            # ---- per-(scol,head) quant scales ----
            mx8 = cpool.tile([128, 8, 1], f32, tag="mx8")  # 0:4 max, 4:8 min
            nc.vector.tensor_reduce(out=mx8[:, 0:4], in_=rot.rearrange(
                "p a h d -> p (a h) d"), axis=X, op=A.max)
            nc.vector.tensor_reduce(out=mx8[:, 4:8], in_=rot.rearrange(
                "p a h d -> p (a h) d"), axis=X, op=A.min)
            ts(mx8[:, 0:4], mx8[:, 0:4], 1e-5, None, A.max)
            ts(mx8[:, 4:8], mx8[:, 4:8], -1e-5, None, A.min)
            rcp8 = cpool.tile([128, 8, 1], f32, tag="rcp8")
            nc.vector.reciprocal(out=rcp8, in_=mx8)
            msc = cpool.tile([128, 8, 1], f16, tag="msc")   # 127/max, 127/min
            ts(msc, rcp8, 127.0)
            qsc = cpool.tile([128, 8, 1], f16, tag="qsc")   # max*al/127, min*al/127
            for half in range(2):
                nc.vector.scalar_tensor_tensor(
                    out=qsc[:, 4 * half:4 * half + 4], in0=mx8[:, 4 * half:4 * half + 4],
                    scalar=1.0 / 127.0, in1=al4, op0=A.mult, op1=A.mult)

            # ---- two-branch fake-quant application ----
            qq = cpool.tile([128, 2, 2, 128], f16, tag="qq")
            tb = cpool.tile([128, 2, 2, 128], f16, tag="tb")
            for s, eng in ((0, nc.vector), (1, nc.gpsimd)):
                pslc = msc[:, 2 * s:2 * s + 2].to_broadcast([128, HLOC, 128])
                nslc = msc[:, 4 + 2 * s:6 + 2 * s].to_broadcast([128, HLOC, 128])
                pq = qsc[:, 2 * s:2 * s + 2].to_broadcast([128, HLOC, 128])
                nq = qsc[:, 4 + 2 * s:6 + 2 * s].to_broadcast([128, HLOC, 128])
                if eng is nc.vector:  # STT is DVE-only
                    eng.scalar_tensor_tensor(out=qq[:, s], in0=rot[:, s],
                                             scalar=0.0, in1=pslc,
                                             op0=A.max, op1=A.mult)
                    eng.scalar_tensor_tensor(out=tb[:, s], in0=rot[:, s],
                                             scalar=0.0, in1=nslc,
                                             op0=A.min, op1=A.mult)
                else:
                    ts(qq[:, s], rot[:, s], 0.0, None, A.max, eng=eng)
                    eng.tensor_tensor(out=qq[:, s], in0=qq[:, s], in1=pslc,
                                      op=A.mult)
                    ts(tb[:, s], rot[:, s], 0.0, None, A.min, eng=eng)
                    eng.tensor_tensor(out=tb[:, s], in0=tb[:, s], in1=nslc,
                                      op=A.mult)
                ts(qq[:, s], qq[:, s], MAGIC16, MAGIC16, A.add, A.subtract, eng=eng)
                eng.tensor_tensor(out=qq[:, s], in0=qq[:, s], in1=pq, op=A.mult)
                ts(tb[:, s], tb[:, s], MAGIC16, MAGIC16, A.add, A.subtract, eng=eng)
                eng.tensor_tensor(out=tb[:, s], in0=tb[:, s], in1=nq, op=A.mult)
                eng.tensor_tensor(out=qq[:, s], in0=qq[:, s], in1=tb[:, s], op=A.add)
            if DEBUG:
                nc.sync.dma_start(out=dbg_qq[i * 128:(i + 1) * 128, :],
                                  in_=qq.rearrange("p a h d -> p (a h d)"))

            qf = qq.rearrange("p a h d -> p (a h d)")
            nc.sync.dma_start_transpose(out=qT[:, i, :, :], in_=qf[:, 0:256])
            nc.sync.dma_start_transpose(out=kT[:, i, :, :], in_=qf[:, 256:512])

        def attn_strip(J, h):
            yps = psY.tile([128, 512], f32, tag="y")
            dps = psD.tile([1, 512], f32, tag="den")
            nblk = 4 * J + 4
            for i in range(nblk):
                st = psS.tile([128, 512], f32, tag="s")
                nc.tensor.matmul(st, kT[:, i, h, :], qT[:, 4 * J:4 * J + 4, h, :],
                                 start=True, stop=True)
                lo = max(0, 128 * (i - 4 * J))
                E = epool.tile([128, 512], f16, tag="E")
                nc.scalar.activation(E[:, lo:512], st[:, lo:512], AF.Exp,
                                     scale=ATTN_SCALE, bias=EXP_SHIFT)
                if i >= 4 * J:
                    nc.gpsimd.affine_select(
                        out=E[:, lo:lo + 128], in_=E[:, lo:lo + 128],
                        compare_op=A.is_ge, fill=0.0, base=0,
                        pattern=[[1, 128]], channel_multiplier=-1)
                nc.tensor.matmul(yps[:, lo:512], vaug[:, i, h, :], E[:, lo:512],
                                 start=(i == 0), stop=(i == nblk - 1))
                nc.tensor.matmul(dps[:, lo:512], onesC, E[:, lo:512],
                                 start=(i == 0), stop=(i == nblk - 1))
            # gate sigmoid via Exp table: g = 1/(1+exp(-z)); fac = g/den
            eg = spool.tile([1, 512], f32, tag="eg")
            nc.scalar.activation(eg, gateZ[h][:, J * 512:(J + 1) * 512],
                                 AF.Exp, scale=-1.0)
            ts(eg, eg, 1.0, None, A.add)
            nc.vector.tensor_tensor(out=eg, in0=eg, in1=dps, op=A.mult)
            fac32 = spool.tile([1, 512], f32, tag="fac32")
            nc.vector.reciprocal(out=fac32, in_=eg)
            fac16 = spool.tile([1, 512], f16, tag="fac16")
            nc.vector.tensor_copy(out=fac16, in_=fac32)
            fps = psS.tile([128, 512], f32, tag="s")
            nc.tensor.matmul(fps, ones1, fac16, start=True, stop=True)
            facb = spool.tile([128, 512], f16, tag="facb")
            nc.scalar.copy(out=facb, in_=fps)
            nc.vector.tensor_tensor(out=yT[:, h, J, :], in0=yps, in1=facb,
                                    op=A.mult)
            if DEBUG:
                nc.sync.dma_start(out=dbg_g[h:h + 1, J * 512:(J + 1) * 512],
                                  in_=fac16)  # noqa
                nc.sync.dma_start(out=dbg_y[:, h, J * 512:(J + 1) * 512],
                                  in_=yT[:, h, J, :])

        def out_tile(i):
            J, jj = divmod(i, 4)
            osb = opool.tile([128, DIM], f16, tag="osb")
            for half in range(2):
                ops_ = psY.tile([128, 512], f32, tag="y")
                for h in range(HLOC):
                    nc.tensor.matmul(ops_, yT[:, h, J, jj * 128:(jj + 1) * 128],
                                     tau_o[:, h, half * 512:(half + 1) * 512],
                                     start=(h == 0), stop=(h == HLOC - 1))
                if half == 0:
                    nc.vector.tensor_copy(out=osb[:, 0:512], in_=ops_)
                else:
                    nc.scalar.copy(out=osb[:, 512:1024], in_=ops_)
            dst = part if USE_RS else outp
            nc.sync.dma_start(out=dst[i * 128:(i + 1) * 128, :], in_=osb)

        for grp in range(NGRP):
            for i in range(4 * grp, 4 * grp + 4):
                qkv_tile(i)
            for h in range(HLOC):
                attn_strip(grp, h)
            for i in range(4 * grp, 4 * grp + 4):
                out_tile(i)
            if USE_RS:
                nc.gpsimd.collective_compute(
                    "ReduceScatter", A.add,
                    replica_groups=[[0, 1, 2, 3], [4, 5, 6, 7]],
                    ins=[part[512 * grp:512 * (grp + 1), :]],
                    outs=[rs_out[128 * grp:128 * (grp + 1), :]])
                nc.sync.dma_start(out=outp[128 * grp:128 * (grp + 1), :],
                                  in_=rs_out[128 * grp:128 * (grp + 1), :])

    nc.compile()
    return nc


def _host_prep(inputs):
    x = np.asarray(inputs["x"], np.float32)
    ve = np.asarray(inputs["ve"], np.float32)
    lam = np.asarray(inputs["sa_lambdas"], np.float32)
    cos = np.asarray(inputs["cos"], np.float32)
    sin = np.asarray(inputs["sin"], np.float32)
    qkvo = np.asarray(inputs["qkvo_w"], np.float32)
    gw = np.asarray(inputs["gate_w"], np.float32)

    # weight ternary quantization (global scales), exact fp32 mirror of ref
    s_qkv = np.maximum(np.abs(qkvo[:3]).mean((1, 2), dtype=np.float32),
                       np.float32(1e-5)).astype(np.float32)
    s_o = np.float32(max(np.abs(qkvo[3]).mean(dtype=np.float32), np.float32(1e-5)))
    tern_qkv = np.clip(np.round(qkvo[:3] / s_qkv[:, None, None]), -1, 1
                       ).astype(np.int8)
    tern_o = np.clip(np.round(qkvo[3] / s_o), -1, 1).astype(np.int8)

    # x int8 fake-quant codes + per-token scales, exact fp32 mirror of ref
    k8b, scb = [], []
    for b in range(B):
        xb = x[b]
        xpmax = np.maximum(xb.max(-1, keepdims=True), np.float32(1e-5))
        xnmin = np.minimum(xb.min(-1, keepdims=True), np.float32(-1e-5))
        rp = np.round((xb / xpmax) * np.float32(127.0))
        rn = np.round((xb / xnmin) * np.float32(127.0))
        k = np.where(xb >= 0, rp, -rn).astype(np.int8)
        sc = np.concatenate([xpmax / np.float32(127.0),
                             xnmin / np.float32(127.0)], 1)  # [T, 2]
        k8b.append(k)
        # [128, NT, 2]: partition-major pack
        scb.append(np.ascontiguousarray(
            sc.reshape(NT, 128, 2).transpose(1, 0, 2)))

    c2 = np.concatenate([cos, cos], 1).astype(np.float16)   # [T,128]
    s2 = np.concatenate([sin, -sin], 1).astype(np.float16)

    scal = np.empty((128, 8), np.float32)
    scal[:, 0] = scal[:, 1] = s_qkv[0] * s_qkv[0] / np.float32(HD)
    scal[:, 2] = scal[:, 3] = s_qkv[1] * s_qkv[1] / np.float32(HD)
    scal[:, 4] = scal[:, 5] = s_qkv[0]
    scal[:, 6] = scal[:, 7] = s_qkv[1]
    lam128 = np.empty((128, 2), np.float32)
    lam128[:, 0] = lam[0] * s_qkv[2] * s_o
    lam128[:, 1] = 0.0

    in_maps = []
    for c in range(8):
        b, g = divmod(c, 4)
        rows = slice(g * ELOC, (g + 1) * ELOC)
        wq = np.concatenate([tern_qkv[s][rows].T for s in range(3)], axis=1)
        in_maps.append({
            "xb8": k8b[b],
            "sctok": scb[b],
            "veb": (ve[b][:, rows] * (lam[1] * s_o)).astype(np.float16),
            "cosd": c2,
            "sind": s2,
            "wqkv8": np.ascontiguousarray(wq),
            "wo8": np.ascontiguousarray(tern_o.T[rows]),
            "gwT": np.ascontiguousarray(gw[2 * g:2 * g + 2].T).astype(np.float16),
            "scal": scal,
            "lam": lam128,
        })
    return in_maps


def kernel(**inputs):
    from concourse.bass_utils import run_bass_kernel_spmd

    if "nc" not in _CACHE:
        _CACHE["nc"] = _build()
    nc = _CACHE["nc"]
    in_maps = _host_prep(inputs)
    res = run_bass_kernel_spmd(nc, in_maps, core_ids=list(range(8)))
    return _assemble([r["outp"] for r in res.results])


def _assemble(outs):
    outs = [o.astype(np.float32) for o in outs]
    out = np.empty((B, T, DIM), np.float32)
    if USE_RS:
        for b in range(B):
            for g in range(4):
                o = outs[4 * b + g]          # [512, DIM]: grp-major chunks
                for grp in range(NGRP):
                    out[b, 512 * grp + 128 * g:512 * grp + 128 * (g + 1)] = \
                        o[128 * grp:128 * (grp + 1)]
    else:
        for b in range(B):
            out[b] = (outs[4 * b] + outs[4 * b + 1] + outs[4 * b + 2]
                      + outs[4 * b + 3])
    return out


if __name__ == "__main__":
    import reference as R
    inputs = R.setup_inputs()
    out = kernel(**{k: np.asarray(v) for k, v in inputs.items()})
    print(out.shape, out.dtype)
